# revision 29
# baseline (speedup 1.0000x reference)
"""CTDG encoder (exp-decay memory GNN) on 8 Trainium2 NeuronCores.

Strategy (pure node-parallel, per the natural sharding of this module):
- Host: shard the 200k nodes into 8 contiguous ranges of 25000 (padded to
  26624 = 13*2048), route each event (unique_sources row) to its owning
  shard, and permute each shard so event nodes come first.  The event
  region is padded to a uniform multiple of 2048 with identity events
  (msg=0, ts=last_update), so every 2048-node "quad" of device columns is
  either fully "event" or fully "plain".  memory/static_emb/messages are
  pre-transposed to feature-major [128, nodes] (bf16) so the device never
  transposes.
- Device (SPMD, identical program, per-core data):
  Pass A: per-node scalars in pair-row layout [26, 1024] (f32 math):
      decay = exp((lu - ts)/30), rc = 1/(cnt_new + eps),
      ds = (1 - e_lamb) * exp((upd_lu - now)/30)   (as exp(x/30 + bias))
    computed twice: an early "slab" over the first 8 pairs (so quad 0-3
    compute starts while the full pass finishes), then the full range.
    Rows are parked in DRAM (bf16) and fetched per quad as partition-0
    rows (DMA on the scalar queue, so the sync queue never blocks).
  Pass B: for each of 13 quads (2048 nodes):
      rc/ds broadcast to [128,2048] SBUF via GPSIMD partition_broadcast
      (uint32-bitcast), decay broadcast via K=1 bf16 matmuls on PE into
      PSUM, event update + count-normalize + output combine on DVE (bf16
      2x, 2048-wide), two-layer MLP on PE (bf16, 512-wide into 1024-wide
      PSUM tiles), LeakyReLU (+bias) on ACT (1024-wide).
- Host: inverse-permute, upcast, and concatenate shard outputs.
"""

import numpy as np
import ml_dtypes

import concourse.bacc as bacc
import concourse.tile as tile
from concourse import mybir
from concourse.bass_utils import run_bass_kernel_spmd

N_NODES = 200000
D = 128
NCORES = 8
S = N_NODES // NCORES          # 25000 real nodes per core
TILE = 512                     # matmul granularity
PAIR = 1024                    # PSUM / activation granularity
QUAD = 2048                    # elementwise / IO granularity
NP = 26                        # pairs per core
NQ = NP // 2                   # 13 quads per core
S_PAD = NP * PAIR              # 26624
SLAB = 8                       # pairs computed early in pass A
LAMB = 30.0                    # memory-updater decay constant
OUTPUT = 30.0                  # embedding time-decay constant
EPS = 1e-10
SLOPE = 0.01

F32 = mybir.dt.float32
BF16 = mybir.dt.bfloat16
U32 = mybir.dt.uint32
NP_BF16 = ml_dtypes.bfloat16


def _build(NEP, e_lamb, now_time):
    """Build the per-core bass program. NEP = number of event pairs (even)."""
    nc = bacc.Bacc("TRN2", target_bir_lowering=False, debug=False,
                   num_devices=NCORES)
    E_PAD = NEP * PAIR
    NEQ = NEP // 2

    msumT_d = nc.dram_tensor("msumT", [D, S_PAD], BF16, kind="ExternalInput")
    # staticT is pre-scaled by e_lamb on the host (constant folding)
    staticT_d = nc.dram_tensor("staticT", [D, S_PAD], BF16, kind="ExternalInput")
    msgT_d = nc.dram_tensor("msgT", [D, E_PAD], BF16, kind="ExternalInput")
    lu_d = nc.dram_tensor("lu_t", [NP, PAIR], F32, kind="ExternalInput")
    ts_d = nc.dram_tensor("ts_t", [NEP, PAIR], F32, kind="ExternalInput")
    cnt_d = nc.dram_tensor("cnt_t", [NP, PAIR], F32, kind="ExternalInput")
    msgc_d = nc.dram_tensor("msgc_t", [NEP, PAIR], F32, kind="ExternalInput")
    w1a_d = nc.dram_tensor("w1a", [D, D], BF16, kind="ExternalInput")
    w1b_d = nc.dram_tensor("w1b", [D, D], BF16, kind="ExternalInput")
    w2_d = nc.dram_tensor("w2", [D, D], BF16, kind="ExternalInput")
    b1_d = nc.dram_tensor("b1", [D, 1], F32, kind="ExternalInput")
    b2_d = nc.dram_tensor("b2", [D, 1], F32, kind="ExternalInput")
    ones_d = nc.dram_tensor("ones", [1, D], BF16, kind="ExternalInput")
    outT_d = nc.dram_tensor("outT", [D, S_PAD], BF16, kind="ExternalOutput")

    # ds = exp(upd_lu/30 - now/30 + ln(1-e_lamb))
    one_m_el = max(1.0 - float(e_lamb), 1e-38)
    ds_bias = float(np.log(one_m_el) - float(now_time) / OUTPUT)
    inv_out = 1.0 / OUTPUT
    inv_lamb = 1.0 / LAMB

    with tile.TileContext(nc) as tc:
        with (
            tc.tile_pool(name="singles", bufs=1) as singles,
            tc.tile_pool(name="psm", bufs=4, space="PSUM") as psm,
            tc.tile_pool(name="dram", bufs=1, space="DRAM") as dram,
        ):
            # ---- constants ----
            ones = singles.tile([1, D], BF16)
            w1a = singles.tile([D, D], BF16)
            w1b = singles.tile([D, D], BF16)
            w2 = singles.tile([D, D], BF16)
            b1 = singles.tile([D, 1], F32)
            b2 = singles.tile([D, 1], F32)

            # pass-A outputs live in a persistent pool: the scl writes read
            # them after passa's address space is already recycled.
            res = tc.alloc_tile_pool(name="res", bufs=1)

            # ---- pass A ----
            passa = tc.alloc_tile_pool(name="passa", bufs=1)
            lu_t = passa.tile([NP, PAIR], F32)
            ts_t = passa.tile([NEP, PAIR], F32)
            cnt_t = passa.tile([NP, PAIR], F32)
            msgc_t = passa.tile([NEP, PAIR], F32)
            nc.sync.dma_start(lu_t, lu_d[:, :])
            nc.sync.dma_start(ts_t, ts_d[:, :])
            nc.sync.dma_start(cnt_t, cnt_d[:, :])
            nc.sync.dma_start(msgc_t, msgc_d[:, :])
            nc.sync.dma_start(ones, ones_d[:, :])
            nc.sync.dma_start(w1a, w1a_d[:, :])
            nc.sync.dma_start(w1b, w1b_d[:, :])
            nc.sync.dma_start(w2, w2_d[:, :])
            nc.sync.dma_start(b1, b1_d[:, :])
            nc.sync.dma_start(b2, b2_d[:, :])

            dec = res.tile([NEP, PAIR], BF16)      # event decay
            rc = res.tile([NP, PAIR], BF16)        # 1/(cnt+eps)
            ds = res.tile([NP, PAIR], BF16)        # (1-e_lamb)*exp((ulu-now)/30)
            ds_bias_t = res.tile([NP, 1], F32)
            nc.vector.memset(ds_bias_t, ds_bias)
            scl = dram.tile([3, NP, PAIR], BF16)

            def pass_a(n, ne):
                """Compute scalars for pair rows [0:n) (event rows [0:ne))."""
                diff = passa.tile([NEP, PAIR], F32, tag="diff", name="diff")
                nc.vector.tensor_sub(diff[:ne, :], lu_t[:ne, :], ts_t[:ne, :])
                nc.scalar.activation(dec[:ne, :], diff[:ne, :],
                                     mybir.ActivationFunctionType.Exp,
                                     scale=inv_lamb)
                cn = passa.tile([NEP, PAIR], F32, tag="cn", name="cn")
                nc.vector.tensor_mul(cn[:ne, :], cnt_t[:ne, :], dec[:ne, :])
                nc.vector.tensor_add(cn[:ne, :], cn[:ne, :], msgc_t[:ne, :])
                rcf = passa.tile([NP, PAIR], F32, tag="rcf", name="rcf")
                nc.vector.reciprocal_approx_fast(rcf[:n, :], cnt_t[:n, :])
                nc.vector.reciprocal_approx_fast(rcf[:ne, :], cn[:ne, :])
                with nc.allow_low_precision(reason="bf16 rounding of 1/cnt"):
                    nc.gpsimd.tensor_copy(rc[:n, :], rcf[:n, :])
                nc.scalar.activation(ds[:n, :], lu_t[:n, :],
                                     mybir.ActivationFunctionType.Exp,
                                     scale=inv_out, bias=ds_bias_t[:n, :])
                nc.scalar.activation(ds[:ne, :], ts_t[:ne, :],
                                     mybir.ActivationFunctionType.Exp,
                                     scale=inv_out, bias=ds_bias_t[:ne, :])

            def scl_write(r0, r1, er1):
                nc.scalar.dma_start(scl[0, r0:r1, :], rc[r0:r1, :])
                nc.scalar.dma_start(scl[1, r0:r1, :], ds[r0:r1, :])
                if er1 > r0:
                    nc.scalar.dma_start(scl[2, r0:er1, :], dec[r0:er1, :])
                if r1 > er1:
                    nc.scalar.dma_start(scl[2, er1:r1, :], rc[er1:r1, :])

            # early slab: unblock the first quads quickly
            sl = min(SLAB, NP)
            sle = min(SLAB, NEP)
            pass_a(sl, sle)
            scl_write(0, sl, sle)
            # full range (recomputes the slab rows; ops cost the same)
            pass_a(NP, NEP)
            scl_write(sl, NP, max(sl, NEP))
            passa.release()

            # ---- pass B: 13 quads of 2048 nodes ----
            io = tc.alloc_tile_pool(name="io", bufs=3)
            vrows = tc.alloc_tile_pool(name="vrows", bufs=3)
            mid = tc.alloc_tile_pool(name="mid", bufs=3)
            bc = tc.alloc_tile_pool(name="bc", bufs=3)
            for q in range(NQ):
                ev = q < NEQ
                col0 = q * QUAD
                qsl = slice(col0, col0 + QUAD)
                ms_q = io.tile([D, QUAD], BF16, name="ms_q")
                nc.sync.dma_start(ms_q, msumT_d[:, qsl])
                st_q = io.tile([D, QUAD], BF16, name="st_q")
                nc.sync.dma_start(st_q, staticT_d[:, qsl])
                if ev:
                    mg_q = io.tile([D, QUAD], BF16, name="mg_q")
                    nc.sync.dma_start(mg_q, msgT_d[:, qsl])

                # scale rows for this quad: [3 planes][2 pairs][PAIR]
                vch = vrows.tile([1, 3 * QUAD], BF16, name="vch")
                nc.scalar.dma_start(vch, scl[:, 2 * q:2 * q + 2, :])

                rc_bc = bc.tile([D, QUAD], BF16, tag="rcbc", name="rc_bc")
                nc.gpsimd.partition_broadcast(rc_bc.bitcast(U32),
                                              vch[0:1, 0:QUAD].bitcast(U32))

                if ev:
                    dec_bc = bc.tile([D, QUAD], BF16, tag="decbc",
                                     name="dec_bc")
                    nc.gpsimd.partition_broadcast(
                        dec_bc.bitcast(U32),
                        vch[0:1, 2 * QUAD:3 * QUAD].bitcast(U32))
                    m3 = mid.tile([D, QUAD], BF16, tag="m3", name="m3")
                    nc.vector.tensor_mul(m3, ms_q, dec_bc)
                    nc.vector.tensor_add(m3, m3, mg_q)
                    ftop = mid.tile([D, QUAD], BF16, tag="ftop", name="ftop")
                    nc.vector.tensor_mul(ftop, m3, rc_bc)
                    fbot = m3
                else:
                    ftop = mid.tile([D, QUAD], BF16, tag="ftop", name="ftop")
                    nc.vector.tensor_mul(ftop, ms_q, rc_bc)
                    fbot = ms_q

                # W1 matmuls for both pairs back-to-back (one LDWEIGHTS per
                # weight per quad keeps the PE stream dense)
                ps1s = []
                for h in range(2):
                    ps1s.append(psm.tile([D, PAIR], F32, tag="mm",
                                         name="ps1"))
                for h in range(2):
                    for t in range(2):
                        tsl = slice(h * PAIR + t * TILE,
                                    h * PAIR + (t + 1) * TILE)
                        nc.tensor.matmul(ps1s[h][:, t * TILE:(t + 1) * TILE],
                                         w1a, ftop[:, tsl],
                                         start=True, stop=False)
                for h in range(2):
                    for t in range(2):
                        tsl = slice(h * PAIR + t * TILE,
                                    h * PAIR + (t + 1) * TILE)
                        nc.tensor.matmul(ps1s[h][:, t * TILE:(t + 1) * TILE],
                                         w1b, fbot[:, tsl],
                                         start=False, stop=True)
                h2 = mid.tile([D, QUAD], BF16, tag="h2", name="h2")
                for h in range(2):
                    hsl = slice(h * PAIR, (h + 1) * PAIR)
                    h1 = mid.tile([D, PAIR], BF16, tag="h1", name="h1")
                    nc.scalar.activation(h1, ps1s[h],
                                         mybir.ActivationFunctionType.Lrelu,
                                         bias=b1, scale=1.0, alpha=SLOPE)
                    ps2 = psm.tile([D, PAIR], F32, tag="mm", name="ps2")
                    for t in range(2):
                        nc.tensor.matmul(ps2[:, t * TILE:(t + 1) * TILE],
                                         w2, h1[:, t * TILE:(t + 1) * TILE],
                                         start=True, stop=True)
                    nc.scalar.activation(h2[:, hsl], ps2,
                                         mybir.ActivationFunctionType.Lrelu,
                                         bias=b2, scale=1.0, alpha=SLOPE)

                ds_bc = bc.tile([D, QUAD], BF16, tag="dsbc", name="ds_bc")
                nc.gpsimd.partition_broadcast(
                    ds_bc.bitcast(U32), vch[0:1, QUAD:2 * QUAD].bitcast(U32))
                t2 = mid.tile([D, QUAD], BF16, tag="t2", name="t2")
                nc.vector.tensor_mul(t2, h2, ds_bc)
                out_q = io.tile([D, QUAD], BF16, name="out_q")
                nc.vector.tensor_add(out_q, t2, st_q)
                nc.sync.dma_start(outT_d[:, qsl], out_q)

            bc.release()
            mid.release()
            vrows.release()
            io.release()
            res.release()

    nc.compile()
    return nc


def _preprocess(memory, last_update, unique_messages, unique_timestamps,
                static_emb, W1, b1, W2, b2, e_lamb, now_time, unique_sources):
    """Shard + route events + permute; returns (in_maps, perms, NEP)."""
    memory = np.asarray(memory, dtype=np.float32)
    last_update = np.asarray(last_update, dtype=np.float32)
    unique_messages = np.asarray(unique_messages, dtype=np.float32)
    unique_timestamps = np.asarray(unique_timestamps, dtype=np.float32)
    static_emb = np.asarray(static_emb, dtype=np.float32)
    unique_sources = np.asarray(unique_sources)

    owner = unique_sources // S
    order = np.argsort(owner, kind="stable")
    counts = np.bincount(owner, minlength=NCORES)
    starts = np.concatenate([[0], np.cumsum(counts)])
    NEP = int(np.ceil(max(1, counts.max()) / QUAD)) * 2  # even # of pairs
    E_PAD = NEP * PAIR

    w1 = np.asarray(W1, dtype=np.float32)
    w1a = np.ascontiguousarray(w1[:D, :]).astype(NP_BF16)
    w1b = np.ascontiguousarray(w1[D:, :]).astype(NP_BF16)
    w2 = np.ascontiguousarray(np.asarray(W2, dtype=np.float32)).astype(NP_BF16)
    b1c = np.asarray(b1, dtype=np.float32).reshape(D, 1).copy()
    b2c = np.asarray(b2, dtype=np.float32).reshape(D, 1).copy()
    ones = np.ones((1, D), dtype=NP_BF16)

    in_maps = []
    perms = []
    for c in range(NCORES):
        ev_rows = order[starts[c]:starts[c + 1]]
        src_local = unique_sources[ev_rows] - c * S
        E_c = src_local.shape[0]

        is_ev = np.zeros(S, dtype=bool)
        is_ev[src_local] = True
        non_ev = np.nonzero(~is_ev)[0]
        perm = np.concatenate([src_local, non_ev]).astype(np.int64)
        perms.append(perm)

        mem_pad = np.empty((S_PAD, D + 1), dtype=np.float32)
        mem_pad[:S] = memory[c * S:(c + 1) * S][perm]
        mem_pad[S:, :D] = 0.0
        mem_pad[S:, D] = 1.0
        lu_pad = np.zeros(S_PAD, dtype=np.float32)
        lu_pad[:S] = last_update[c * S:(c + 1) * S][perm]
        st_pad = np.zeros((S_PAD, D), dtype=np.float32)
        st_pad[:S] = static_emb[c * S:(c + 1) * S][perm]
        st_pad *= np.float32(e_lamb)   # fold e_lamb into the static table

        msg_full = np.zeros((E_PAD, D + 1), dtype=np.float32)
        msg_full[:E_c] = unique_messages[ev_rows]
        ts_full = np.empty(E_PAD, dtype=np.float32)
        ts_full[:E_c] = unique_timestamps[ev_rows]
        ts_full[E_c:] = lu_pad[E_c:E_PAD]   # identity events: ts = lu, msg = 0

        in_maps.append({
            "msumT": np.ascontiguousarray(mem_pad[:, :D].T).astype(NP_BF16),
            "staticT": np.ascontiguousarray(st_pad.T).astype(NP_BF16),
            "msgT": np.ascontiguousarray(msg_full[:, :D].T).astype(NP_BF16),
            "lu_t": lu_pad.reshape(NP, PAIR).copy(),
            "ts_t": ts_full.reshape(NEP, PAIR).copy(),
            "cnt_t": mem_pad[:, D].reshape(NP, PAIR).copy(),
            "msgc_t": msg_full[:, D].reshape(NEP, PAIR).copy(),
            "w1a": w1a, "w1b": w1b, "w2": w2,
            "b1": b1c, "b2": b2c, "ones": ones,
        })
    return in_maps, perms, NEP


def _run(inputs, trace=False, trace_cores=None):
    in_maps, perms, NEP = _preprocess(**inputs)
    nc = _build(NEP, inputs["e_lamb"], inputs["now_time"])
    res = run_bass_kernel_spmd(nc, in_maps, core_ids=list(range(NCORES)),
                               trace=trace, trace_cores=trace_cores)
    out = np.empty((N_NODES, D), dtype=np.float32)
    for c in range(NCORES):
        out_perm = res.results[c]["outT"].T[:S].astype(np.float32)
        shard = np.empty((S, D), dtype=np.float32)
        shard[perms[c]] = out_perm
        out[c * S:(c + 1) * S] = shard
    return out, res


def kernel(**inputs) -> np.ndarray:
    out, _ = _run(inputs, trace=False)
    return out


# revision 30
# speedup vs baseline: 1.1247x; 1.1247x over previous
"""CTDG encoder (exp-decay memory GNN) on 8 Trainium2 NeuronCores.

Strategy (pure node-parallel, per the natural sharding of this module):
- Host: shard the 200k nodes into 8 contiguous ranges of 25000 (padded to
  26624 = 13*2048), route each event (unique_sources row) to its owning
  shard, and permute each shard so event nodes come first.  The event
  region is padded to a uniform multiple of 2048 with identity events
  (msg=0, ts=last_update), so every 2048-node "quad" of device columns is
  either fully "event" or fully "plain".  memory/static_emb/messages are
  pre-transposed to feature-major [128, nodes] (bf16) so the device never
  transposes.
- Device (SPMD, identical program, per-core data):
  Pass A: per-node scalars in pair-row layout [26, 1024] (f32 math):
      decay = exp((lu - ts)/30), rc = 1/(cnt_new + eps),
      ds = (1 - e_lamb) * exp((upd_lu - now)/30)   (as exp(x/30 + bias))
    computed twice: an early "slab" over the first 8 pairs (so quad 0-3
    compute starts while the full pass finishes), then the full range.
    Rows are parked in DRAM (bf16) and fetched per quad as partition-0
    rows (DMA on the scalar queue, so the sync queue never blocks).
  Pass B: for each of 13 quads (2048 nodes):
      rc/ds broadcast to [128,2048] SBUF via GPSIMD partition_broadcast
      (uint32-bitcast), decay broadcast via K=1 bf16 matmuls on PE into
      PSUM, event update + count-normalize + output combine on DVE (bf16
      2x, 2048-wide), two-layer MLP on PE (bf16, 512-wide into 1024-wide
      PSUM tiles), LeakyReLU (+bias) on ACT (1024-wide).
- Host: inverse-permute, upcast, and concatenate shard outputs.
"""

import numpy as np
import ml_dtypes

import concourse.bacc as bacc
import concourse.tile as tile
from concourse import mybir
from concourse.bass_utils import run_bass_kernel_spmd

N_NODES = 200000
D = 128
NCORES = 8
S = N_NODES // NCORES          # 25000 real nodes per core
TILE = 512                     # matmul granularity
PAIR = 1024                    # PSUM / activation granularity
QUAD = 2048                    # elementwise / IO granularity
NP = 26                        # pairs per core
NQ = NP // 2                   # 13 quads per core
S_PAD = NP * PAIR              # 26624
SLAB = 8                       # pairs computed early in pass A
LAMB = 30.0                    # memory-updater decay constant
OUTPUT = 30.0                  # embedding time-decay constant
EPS = 1e-10
SLOPE = 0.01

F32 = mybir.dt.float32
BF16 = mybir.dt.bfloat16
U32 = mybir.dt.uint32
NP_BF16 = ml_dtypes.bfloat16


def _build(NEP, e_lamb, now_time):
    """Build the per-core bass program. NEP = number of event pairs (even)."""
    nc = bacc.Bacc("TRN2", target_bir_lowering=False, debug=False,
                   num_devices=NCORES)
    E_PAD = NEP * PAIR
    NEQ = NEP // 2

    msumT_d = nc.dram_tensor("msumT", [D, S_PAD], BF16, kind="ExternalInput")
    # staticT is pre-scaled by e_lamb on the host (constant folding)
    staticT_d = nc.dram_tensor("staticT", [D, S_PAD], BF16, kind="ExternalInput")
    msgT_d = nc.dram_tensor("msgT", [D, E_PAD], BF16, kind="ExternalInput")
    lu_d = nc.dram_tensor("lu_t", [NP, PAIR], F32, kind="ExternalInput")
    ts_d = nc.dram_tensor("ts_t", [NEP, PAIR], F32, kind="ExternalInput")
    cnt_d = nc.dram_tensor("cnt_t", [NP, PAIR], F32, kind="ExternalInput")
    msgc_d = nc.dram_tensor("msgc_t", [NEP, PAIR], F32, kind="ExternalInput")
    w1a_d = nc.dram_tensor("w1a", [D, D], BF16, kind="ExternalInput")
    w1b_d = nc.dram_tensor("w1b", [D, D], BF16, kind="ExternalInput")
    w2_d = nc.dram_tensor("w2", [D, D], BF16, kind="ExternalInput")
    b1_d = nc.dram_tensor("b1", [D, 1], F32, kind="ExternalInput")
    b2_d = nc.dram_tensor("b2", [D, 1], F32, kind="ExternalInput")
    ones_d = nc.dram_tensor("ones", [1, D], BF16, kind="ExternalInput")
    outT_d = nc.dram_tensor("outT", [D, S_PAD], BF16, kind="ExternalOutput")

    # ds = exp(upd_lu/30 - now/30 + ln(1-e_lamb))
    one_m_el = max(1.0 - float(e_lamb), 1e-38)
    ds_bias = float(np.log(one_m_el) - float(now_time) / OUTPUT)
    inv_out = 1.0 / OUTPUT
    inv_lamb = 1.0 / LAMB

    with tile.TileContext(nc) as tc:
        with (
            tc.tile_pool(name="singles", bufs=1) as singles,
            tc.tile_pool(name="psm", bufs=4, space="PSUM") as psm,
            tc.tile_pool(name="dram", bufs=1, space="DRAM") as dram,
        ):
            # ---- constants ----
            ones = singles.tile([1, D], BF16)
            w1a = singles.tile([D, D], BF16)
            w1b = singles.tile([D, D], BF16)
            w2 = singles.tile([D, D], BF16)
            b1 = singles.tile([D, 1], F32)
            b2 = singles.tile([D, 1], F32)

            # pass-A outputs live in a persistent pool: the scl writes read
            # them after passa's address space is already recycled.
            res = tc.alloc_tile_pool(name="res", bufs=1)

            # ---- pass A ----
            passa = tc.alloc_tile_pool(name="passa", bufs=1)
            lu_t = passa.tile([NP, PAIR], F32)
            ts_t = passa.tile([NEP, PAIR], F32)
            cnt_t = passa.tile([NP, PAIR], F32)
            msgc_t = passa.tile([NEP, PAIR], F32)
            nc.sync.dma_start(lu_t, lu_d[:, :])
            nc.sync.dma_start(ts_t, ts_d[:, :])
            nc.sync.dma_start(cnt_t, cnt_d[:, :])
            nc.sync.dma_start(msgc_t, msgc_d[:, :])
            nc.sync.dma_start(ones, ones_d[:, :])
            nc.sync.dma_start(w1a, w1a_d[:, :])
            nc.sync.dma_start(w1b, w1b_d[:, :])
            nc.sync.dma_start(w2, w2_d[:, :])
            nc.sync.dma_start(b1, b1_d[:, :])
            nc.sync.dma_start(b2, b2_d[:, :])

            dec = res.tile([NEP, PAIR], BF16)      # event decay
            rc = res.tile([NP, PAIR], BF16)        # 1/(cnt+eps)
            ds = res.tile([NP, PAIR], BF16)        # (1-e_lamb)*exp((ulu-now)/30)
            ds_bias_t = res.tile([NP, 1], F32)
            nc.vector.memset(ds_bias_t, ds_bias)
            scl = dram.tile([3, NP, PAIR], BF16)

            def pass_a(n, ne):
                """Compute scalars for pair rows [0:n) (event rows [0:ne))."""
                diff = passa.tile([NEP, PAIR], F32, tag="diff", name="diff")
                nc.vector.tensor_sub(diff[:ne, :], lu_t[:ne, :], ts_t[:ne, :])
                nc.scalar.activation(dec[:ne, :], diff[:ne, :],
                                     mybir.ActivationFunctionType.Exp,
                                     scale=inv_lamb)
                cn = passa.tile([NEP, PAIR], F32, tag="cn", name="cn")
                nc.vector.tensor_mul(cn[:ne, :], cnt_t[:ne, :], dec[:ne, :])
                nc.vector.tensor_add(cn[:ne, :], cn[:ne, :], msgc_t[:ne, :])
                rcf = passa.tile([NP, PAIR], F32, tag="rcf", name="rcf")
                nc.vector.reciprocal_approx_fast(rcf[:n, :], cnt_t[:n, :])
                nc.vector.reciprocal_approx_fast(rcf[:ne, :], cn[:ne, :])
                with nc.allow_low_precision(reason="bf16 rounding of 1/cnt"):
                    nc.vector.tensor_copy(rc[:n, :], rcf[:n, :])
                nc.scalar.activation(ds[:n, :], lu_t[:n, :],
                                     mybir.ActivationFunctionType.Exp,
                                     scale=inv_out, bias=ds_bias_t[:n, :])
                nc.scalar.activation(ds[:ne, :], ts_t[:ne, :],
                                     mybir.ActivationFunctionType.Exp,
                                     scale=inv_out, bias=ds_bias_t[:ne, :])

            def scl_write(r0, r1, er1):
                nc.scalar.dma_start(scl[0, r0:r1, :], rc[r0:r1, :])
                nc.scalar.dma_start(scl[1, r0:r1, :], ds[r0:r1, :])
                if er1 > r0:
                    nc.scalar.dma_start(scl[2, r0:er1, :], dec[r0:er1, :])
                if r1 > er1:
                    nc.scalar.dma_start(scl[2, er1:r1, :], rc[er1:r1, :])

            # early slab: unblock the first quads quickly
            sl = min(SLAB, NP)
            sle = min(SLAB, NEP)
            pass_a(sl, sle)
            scl_write(0, sl, sle)
            # full range (recomputes the slab rows; ops cost the same)
            pass_a(NP, NEP)
            scl_write(sl, NP, max(sl, NEP))
            passa.release()

            # ---- pass B: 13 quads of 2048 nodes ----
            io = tc.alloc_tile_pool(name="io", bufs=3)
            vrows = tc.alloc_tile_pool(name="vrows", bufs=3)
            mid = tc.alloc_tile_pool(name="mid", bufs=3)
            bc = tc.alloc_tile_pool(name="bc", bufs=3)
            for q in range(NQ):
                ev = q < NEQ
                col0 = q * QUAD
                qsl = slice(col0, col0 + QUAD)
                ms_q = io.tile([D, QUAD], BF16, name="ms_q")
                nc.sync.dma_start(ms_q, msumT_d[:, qsl])
                st_q = io.tile([D, QUAD], BF16, name="st_q")
                nc.sync.dma_start(st_q, staticT_d[:, qsl])
                if ev:
                    mg_q = io.tile([D, QUAD], BF16, name="mg_q")
                    nc.sync.dma_start(mg_q, msgT_d[:, qsl])

                # scale rows for this quad: [3 planes][2 pairs][PAIR]
                vch = vrows.tile([1, 3 * QUAD], BF16, name="vch")
                nc.scalar.dma_start(vch, scl[:, 2 * q:2 * q + 2, :])

                rc_bc = bc.tile([D, QUAD], BF16, tag="rcbc", name="rc_bc")
                nc.gpsimd.partition_broadcast(rc_bc.bitcast(U32),
                                              vch[0:1, 0:QUAD].bitcast(U32))
                ds_bc = bc.tile([D, QUAD], BF16, tag="dsbc", name="ds_bc")
                nc.gpsimd.partition_broadcast(
                    ds_bc.bitcast(U32), vch[0:1, QUAD:2 * QUAD].bitcast(U32))

                if ev:
                    dec_bc = bc.tile([D, QUAD], BF16, tag="decbc",
                                     name="dec_bc")
                    nc.gpsimd.partition_broadcast(
                        dec_bc.bitcast(U32),
                        vch[0:1, 2 * QUAD:3 * QUAD].bitcast(U32))
                    m3 = mid.tile([D, QUAD], BF16, tag="m3", name="m3")
                    nc.vector.tensor_mul(m3, ms_q, dec_bc)
                    nc.vector.tensor_add(m3, m3, mg_q)
                    ftop = mid.tile([D, QUAD], BF16, tag="ftop", name="ftop")
                    nc.vector.tensor_mul(ftop, m3, rc_bc)
                    fbot = m3
                else:
                    ftop = mid.tile([D, QUAD], BF16, tag="ftop", name="ftop")
                    nc.vector.tensor_mul(ftop, ms_q, rc_bc)
                    fbot = ms_q

                # W1 matmuls for both pairs back-to-back (one LDWEIGHTS per
                # weight per quad keeps the PE stream dense)
                ps1s = []
                for h in range(2):
                    ps1s.append(psm.tile([D, PAIR], F32, tag="mm",
                                         name="ps1"))
                for h in range(2):
                    for t in range(2):
                        tsl = slice(h * PAIR + t * TILE,
                                    h * PAIR + (t + 1) * TILE)
                        nc.tensor.matmul(ps1s[h][:, t * TILE:(t + 1) * TILE],
                                         w1a, ftop[:, tsl],
                                         start=True, stop=False)
                for h in range(2):
                    for t in range(2):
                        tsl = slice(h * PAIR + t * TILE,
                                    h * PAIR + (t + 1) * TILE)
                        nc.tensor.matmul(ps1s[h][:, t * TILE:(t + 1) * TILE],
                                         w1b, fbot[:, tsl],
                                         start=False, stop=True)
                h2 = mid.tile([D, QUAD], BF16, tag="h2", name="h2")
                for h in range(2):
                    hsl = slice(h * PAIR, (h + 1) * PAIR)
                    h1 = mid.tile([D, PAIR], BF16, tag="h1", name="h1")
                    nc.scalar.activation(h1, ps1s[h],
                                         mybir.ActivationFunctionType.Lrelu,
                                         bias=b1, scale=1.0, alpha=SLOPE)
                    ps2 = psm.tile([D, PAIR], F32, tag="mm", name="ps2")
                    for t in range(2):
                        nc.tensor.matmul(ps2[:, t * TILE:(t + 1) * TILE],
                                         w2, h1[:, t * TILE:(t + 1) * TILE],
                                         start=True, stop=True)
                    nc.scalar.activation(h2[:, hsl], ps2,
                                         mybir.ActivationFunctionType.Lrelu,
                                         bias=b2, scale=1.0, alpha=SLOPE)

                t2 = mid.tile([D, QUAD], BF16, tag="t2", name="t2")
                nc.vector.tensor_mul(t2, h2, ds_bc)
                out_q = io.tile([D, QUAD], BF16, name="out_q")
                nc.vector.tensor_add(out_q, t2, st_q)
                nc.sync.dma_start(outT_d[:, qsl], out_q)

            bc.release()
            mid.release()
            vrows.release()
            io.release()
            res.release()

    nc.compile()
    return nc


def _preprocess(memory, last_update, unique_messages, unique_timestamps,
                static_emb, W1, b1, W2, b2, e_lamb, now_time, unique_sources):
    """Shard + route events + permute; returns (in_maps, perms, NEP)."""
    memory = np.asarray(memory, dtype=np.float32)
    last_update = np.asarray(last_update, dtype=np.float32)
    unique_messages = np.asarray(unique_messages, dtype=np.float32)
    unique_timestamps = np.asarray(unique_timestamps, dtype=np.float32)
    static_emb = np.asarray(static_emb, dtype=np.float32)
    unique_sources = np.asarray(unique_sources)

    owner = unique_sources // S
    order = np.argsort(owner, kind="stable")
    counts = np.bincount(owner, minlength=NCORES)
    starts = np.concatenate([[0], np.cumsum(counts)])
    NEP = int(np.ceil(max(1, counts.max()) / QUAD)) * 2  # even # of pairs
    E_PAD = NEP * PAIR

    w1 = np.asarray(W1, dtype=np.float32)
    w1a = np.ascontiguousarray(w1[:D, :]).astype(NP_BF16)
    w1b = np.ascontiguousarray(w1[D:, :]).astype(NP_BF16)
    w2 = np.ascontiguousarray(np.asarray(W2, dtype=np.float32)).astype(NP_BF16)
    b1c = np.asarray(b1, dtype=np.float32).reshape(D, 1).copy()
    b2c = np.asarray(b2, dtype=np.float32).reshape(D, 1).copy()
    ones = np.ones((1, D), dtype=NP_BF16)

    in_maps = []
    perms = []
    for c in range(NCORES):
        ev_rows = order[starts[c]:starts[c + 1]]
        src_local = unique_sources[ev_rows] - c * S
        E_c = src_local.shape[0]

        is_ev = np.zeros(S, dtype=bool)
        is_ev[src_local] = True
        non_ev = np.nonzero(~is_ev)[0]
        perm = np.concatenate([src_local, non_ev]).astype(np.int64)
        perms.append(perm)

        mem_pad = np.empty((S_PAD, D + 1), dtype=np.float32)
        mem_pad[:S] = memory[c * S:(c + 1) * S][perm]
        mem_pad[S:, :D] = 0.0
        mem_pad[S:, D] = 1.0
        lu_pad = np.zeros(S_PAD, dtype=np.float32)
        lu_pad[:S] = last_update[c * S:(c + 1) * S][perm]
        st_pad = np.zeros((S_PAD, D), dtype=np.float32)
        st_pad[:S] = static_emb[c * S:(c + 1) * S][perm]
        st_pad *= np.float32(e_lamb)   # fold e_lamb into the static table

        msg_full = np.zeros((E_PAD, D + 1), dtype=np.float32)
        msg_full[:E_c] = unique_messages[ev_rows]
        ts_full = np.empty(E_PAD, dtype=np.float32)
        ts_full[:E_c] = unique_timestamps[ev_rows]
        ts_full[E_c:] = lu_pad[E_c:E_PAD]   # identity events: ts = lu, msg = 0

        in_maps.append({
            "msumT": np.ascontiguousarray(mem_pad[:, :D].T).astype(NP_BF16),
            "staticT": np.ascontiguousarray(st_pad.T).astype(NP_BF16),
            "msgT": np.ascontiguousarray(msg_full[:, :D].T).astype(NP_BF16),
            "lu_t": lu_pad.reshape(NP, PAIR).copy(),
            "ts_t": ts_full.reshape(NEP, PAIR).copy(),
            "cnt_t": mem_pad[:, D].reshape(NP, PAIR).copy(),
            "msgc_t": msg_full[:, D].reshape(NEP, PAIR).copy(),
            "w1a": w1a, "w1b": w1b, "w2": w2,
            "b1": b1c, "b2": b2c, "ones": ones,
        })
    return in_maps, perms, NEP


def _run(inputs, trace=False, trace_cores=None):
    in_maps, perms, NEP = _preprocess(**inputs)
    nc = _build(NEP, inputs["e_lamb"], inputs["now_time"])
    res = run_bass_kernel_spmd(nc, in_maps, core_ids=list(range(NCORES)),
                               trace=trace, trace_cores=trace_cores)
    out = np.empty((N_NODES, D), dtype=np.float32)
    for c in range(NCORES):
        out_perm = res.results[c]["outT"].T[:S].astype(np.float32)
        shard = np.empty((S, D), dtype=np.float32)
        shard[perms[c]] = out_perm
        out[c * S:(c + 1) * S] = shard
    return out, res


def kernel(**inputs) -> np.ndarray:
    out, _ = _run(inputs, trace=False)
    return out


# revision 31
# speedup vs baseline: 1.1475x; 1.0203x over previous
"""CTDG encoder (exp-decay memory GNN) on 8 Trainium2 NeuronCores.

Strategy (pure node-parallel, per the natural sharding of this module):
- Host: shard the 200k nodes into 8 contiguous ranges of 25000 (padded to
  26624 = 13*2048), route each event (unique_sources row) to its owning
  shard, and permute each shard so event nodes come first.  The event
  region is padded to a uniform multiple of 2048 with identity events
  (msg=0, ts=last_update), so every 2048-node "quad" of device columns is
  either fully "event" or fully "plain".  memory/static_emb/messages are
  pre-transposed to feature-major [128, nodes] (bf16) so the device never
  transposes.
- Device (SPMD, identical program, per-core data):
  Pass A: per-node scalars in pair-row layout [26, 1024] (f32 math):
      decay = exp((lu - ts)/30), rc = 1/(cnt_new + eps),
      ds = (1 - e_lamb) * exp((upd_lu - now)/30)   (as exp(x/30 + bias))
    computed twice: an early "slab" over the first 8 pairs (so quad 0-3
    compute starts while the full pass finishes), then the full range.
    Rows are parked in DRAM (bf16) and fetched per quad as partition-0
    rows (DMA on the scalar queue, so the sync queue never blocks).
  Pass B: for each of 13 quads (2048 nodes):
      rc/ds broadcast to [128,2048] SBUF via GPSIMD partition_broadcast
      (uint32-bitcast), decay broadcast via K=1 bf16 matmuls on PE into
      PSUM, event update + count-normalize + output combine on DVE (bf16
      2x, 2048-wide), two-layer MLP on PE (bf16, 512-wide into 1024-wide
      PSUM tiles), LeakyReLU (+bias) on ACT (1024-wide).
- Host: inverse-permute, upcast, and concatenate shard outputs.
"""

import numpy as np
import ml_dtypes

import concourse.bacc as bacc
import concourse.tile as tile
from concourse import mybir
from concourse.bass_utils import run_bass_kernel_spmd

N_NODES = 200000
D = 128
NCORES = 8
S = N_NODES // NCORES          # 25000 real nodes per core
TILE = 512                     # matmul granularity
PAIR = 1024                    # PSUM / activation granularity
QUAD = 2048                    # elementwise / IO granularity
NP = 26                        # pairs per core
NQ = NP // 2                   # 13 quads per core
S_PAD = NP * PAIR              # 26624
SLAB = 8                       # pairs computed early in pass A
LAMB = 30.0                    # memory-updater decay constant
OUTPUT = 30.0                  # embedding time-decay constant
EPS = 1e-10
SLOPE = 0.01

F32 = mybir.dt.float32
BF16 = mybir.dt.bfloat16
U32 = mybir.dt.uint32
NP_BF16 = ml_dtypes.bfloat16


def _build(NEP, e_lamb, now_time):
    """Build the per-core bass program. NEP = number of event pairs (even)."""
    nc = bacc.Bacc("TRN2", target_bir_lowering=False, debug=False,
                   num_devices=NCORES)
    E_PAD = NEP * PAIR
    NEQ = NEP // 2

    msumT_d = nc.dram_tensor("msumT", [D, S_PAD], BF16, kind="ExternalInput")
    # staticT is pre-scaled by e_lamb on the host (constant folding)
    staticT_d = nc.dram_tensor("staticT", [D, S_PAD], BF16, kind="ExternalInput")
    msgT_d = nc.dram_tensor("msgT", [D, E_PAD], BF16, kind="ExternalInput")
    lu_d = nc.dram_tensor("lu_t", [NP, PAIR], F32, kind="ExternalInput")
    ts_d = nc.dram_tensor("ts_t", [NEP, PAIR], F32, kind="ExternalInput")
    cnt_d = nc.dram_tensor("cnt_t", [NP, PAIR], F32, kind="ExternalInput")
    msgc_d = nc.dram_tensor("msgc_t", [NEP, PAIR], F32, kind="ExternalInput")
    w1a_d = nc.dram_tensor("w1a", [D, D], BF16, kind="ExternalInput")
    w1b_d = nc.dram_tensor("w1b", [D, D], BF16, kind="ExternalInput")
    w2_d = nc.dram_tensor("w2", [D, D], BF16, kind="ExternalInput")
    b1_d = nc.dram_tensor("b1", [D, 1], F32, kind="ExternalInput")
    b2_d = nc.dram_tensor("b2", [D, 1], F32, kind="ExternalInput")
    ones_d = nc.dram_tensor("ones", [1, D], BF16, kind="ExternalInput")
    outT_d = nc.dram_tensor("outT", [D, S_PAD], BF16, kind="ExternalOutput")

    # ds = exp(upd_lu/30 - now/30 + ln(1-e_lamb))
    one_m_el = max(1.0 - float(e_lamb), 1e-38)
    ds_bias = float(np.log(one_m_el) - float(now_time) / OUTPUT)
    inv_out = 1.0 / OUTPUT
    inv_lamb = 1.0 / LAMB

    with tile.TileContext(nc) as tc:
        with (
            tc.tile_pool(name="singles", bufs=1) as singles,
            tc.tile_pool(name="psm", bufs=4, space="PSUM") as psm,
            tc.tile_pool(name="dram", bufs=1, space="DRAM") as dram,
        ):
            # ---- constants ----
            ones = singles.tile([1, D], BF16)
            w1a = singles.tile([D, D], BF16)
            w1b = singles.tile([D, D], BF16)
            w2 = singles.tile([D, D], BF16)
            b1 = singles.tile([D, 1], F32)
            b2 = singles.tile([D, 1], F32)

            # pass-A outputs live in a persistent pool: the scl writes read
            # them after passa's address space is already recycled.
            res = tc.alloc_tile_pool(name="res", bufs=1)

            # ---- pass A ----
            passa = tc.alloc_tile_pool(name="passa", bufs=1)
            lu_t = passa.tile([NP, PAIR], F32)
            ts_t = passa.tile([NEP, PAIR], F32)
            cnt_t = passa.tile([NP, PAIR], F32)
            msgc_t = passa.tile([NEP, PAIR], F32)
            nc.sync.dma_start(lu_t, lu_d[:, :])
            nc.sync.dma_start(ts_t, ts_d[:, :])
            nc.sync.dma_start(cnt_t, cnt_d[:, :])
            nc.sync.dma_start(msgc_t, msgc_d[:, :])
            nc.sync.dma_start(ones, ones_d[:, :])
            nc.sync.dma_start(w1a, w1a_d[:, :])
            nc.sync.dma_start(w1b, w1b_d[:, :])
            nc.sync.dma_start(w2, w2_d[:, :])
            nc.sync.dma_start(b1, b1_d[:, :])
            nc.sync.dma_start(b2, b2_d[:, :])

            dec = res.tile([NEP, PAIR], BF16)      # event decay
            rc = res.tile([NP, PAIR], BF16)        # 1/(cnt+eps)
            ds = res.tile([NP, PAIR], BF16)        # (1-e_lamb)*exp((ulu-now)/30)
            ds_bias_t = res.tile([NP, 1], F32)
            nc.vector.memset(ds_bias_t, ds_bias)
            scl = dram.tile([3, NP, PAIR], BF16)

            def pass_a(n, ne):
                """Compute scalars for pair rows [0:n) (event rows [0:ne))."""
                diff = passa.tile([NEP, PAIR], F32, tag="diff", name="diff")
                nc.vector.tensor_sub(diff[:ne, :], lu_t[:ne, :], ts_t[:ne, :])
                nc.scalar.activation(dec[:ne, :], diff[:ne, :],
                                     mybir.ActivationFunctionType.Exp,
                                     scale=inv_lamb)
                cn = passa.tile([NEP, PAIR], F32, tag="cn", name="cn")
                nc.vector.tensor_mul(cn[:ne, :], cnt_t[:ne, :], dec[:ne, :])
                nc.vector.tensor_add(cn[:ne, :], cn[:ne, :], msgc_t[:ne, :])
                rcf = passa.tile([NP, PAIR], F32, tag="rcf", name="rcf")
                nc.vector.reciprocal_approx_fast(rcf[:n, :], cnt_t[:n, :])
                nc.vector.reciprocal_approx_fast(rcf[:ne, :], cn[:ne, :])
                with nc.allow_low_precision(reason="bf16 rounding of 1/cnt"):
                    nc.vector.tensor_copy(rc[:n, :], rcf[:n, :])
                nc.scalar.activation(ds[:n, :], lu_t[:n, :],
                                     mybir.ActivationFunctionType.Exp,
                                     scale=inv_out, bias=ds_bias_t[:n, :])
                nc.scalar.activation(ds[:ne, :], ts_t[:ne, :],
                                     mybir.ActivationFunctionType.Exp,
                                     scale=inv_out, bias=ds_bias_t[:ne, :])

            def scl_write(r0, r1, er1):
                nc.scalar.dma_start(scl[0, r0:r1, :], rc[r0:r1, :])
                nc.scalar.dma_start(scl[1, r0:r1, :], ds[r0:r1, :])
                if er1 > r0:
                    nc.scalar.dma_start(scl[2, r0:er1, :], dec[r0:er1, :])
                if r1 > er1:
                    nc.scalar.dma_start(scl[2, er1:r1, :], rc[er1:r1, :])

            # early slab: unblock the first quads quickly
            sl = min(SLAB, NP)
            sle = min(SLAB, NEP)
            pass_a(sl, sle)
            scl_write(0, sl, sle)
            # full range (recomputes the slab rows; ops cost the same)
            pass_a(NP, NEP)
            scl_write(sl, NP, max(sl, NEP))
            passa.release()

            # ---- pass B: 13 quads of 2048 nodes ----
            io = tc.alloc_tile_pool(name="io", bufs=3)
            vrows = tc.alloc_tile_pool(name="vrows", bufs=2)
            mid = tc.alloc_tile_pool(name="mid", bufs=3)
            bc = tc.alloc_tile_pool(name="bc", bufs=4)
            qorder = []
            a, b_ = 0, NEQ
            while a < NEQ or b_ < NQ:
                if a < NEQ:
                    qorder.append(a); a += 1
                if b_ < NQ:
                    qorder.append(b_); b_ += 1
            for q in qorder:
                ev = q < NEQ
                col0 = q * QUAD
                qsl = slice(col0, col0 + QUAD)
                ms_q = io.tile([D, QUAD], BF16, name="ms_q")
                nc.sync.dma_start(ms_q, msumT_d[:, qsl])
                st_q = io.tile([D, QUAD], BF16, name="st_q")
                nc.sync.dma_start(st_q, staticT_d[:, qsl])
                if ev:
                    mg_q = io.tile([D, QUAD], BF16, name="mg_q")
                    nc.sync.dma_start(mg_q, msgT_d[:, qsl])

                # scale rows for this quad: [3 planes][2 pairs][PAIR]
                vch = vrows.tile([1, 3 * QUAD], BF16, name="vch")
                nc.scalar.dma_start(vch, scl[:, 2 * q:2 * q + 2, :])

                rc_bc = bc.tile([D, QUAD], BF16, tag="rcbc", name="rc_bc")
                nc.gpsimd.partition_broadcast(rc_bc.bitcast(U32),
                                              vch[0:1, 0:QUAD].bitcast(U32))
                ds_bc = bc.tile([D, QUAD], BF16, tag="dsbc", name="ds_bc")
                nc.gpsimd.partition_broadcast(
                    ds_bc.bitcast(U32), vch[0:1, QUAD:2 * QUAD].bitcast(U32))

                if ev:
                    dec_bc = bc.tile([D, QUAD], BF16, tag="decbc",
                                     name="dec_bc")
                    nc.gpsimd.partition_broadcast(
                        dec_bc.bitcast(U32),
                        vch[0:1, 2 * QUAD:3 * QUAD].bitcast(U32))
                    m3 = mid.tile([D, QUAD], BF16, tag="m3", name="m3")
                    nc.vector.tensor_mul(m3, ms_q, dec_bc)
                    nc.vector.tensor_add(m3, m3, mg_q)
                    ftop = mid.tile([D, QUAD], BF16, tag="ftop", name="ftop")
                    nc.vector.tensor_mul(ftop, m3, rc_bc)
                    fbot = m3
                else:
                    ftop = mid.tile([D, QUAD], BF16, tag="ftop", name="ftop")
                    nc.vector.tensor_mul(ftop, ms_q, rc_bc)
                    fbot = ms_q

                # W1 matmuls for both pairs back-to-back (one LDWEIGHTS per
                # weight per quad keeps the PE stream dense)
                ps1s = []
                for h in range(2):
                    ps1s.append(psm.tile([D, PAIR], F32, tag="mm",
                                         name="ps1"))
                for h in range(2):
                    for t in range(2):
                        tsl = slice(h * PAIR + t * TILE,
                                    h * PAIR + (t + 1) * TILE)
                        nc.tensor.matmul(ps1s[h][:, t * TILE:(t + 1) * TILE],
                                         w1a, ftop[:, tsl],
                                         start=True, stop=False)
                for h in range(2):
                    for t in range(2):
                        tsl = slice(h * PAIR + t * TILE,
                                    h * PAIR + (t + 1) * TILE)
                        nc.tensor.matmul(ps1s[h][:, t * TILE:(t + 1) * TILE],
                                         w1b, fbot[:, tsl],
                                         start=False, stop=True)
                h2 = mid.tile([D, QUAD], BF16, tag="h2", name="h2")
                for h in range(2):
                    hsl = slice(h * PAIR, (h + 1) * PAIR)
                    h1 = mid.tile([D, PAIR], BF16, tag="h1", name="h1")
                    nc.scalar.activation(h1, ps1s[h],
                                         mybir.ActivationFunctionType.Lrelu,
                                         bias=b1, scale=1.0, alpha=SLOPE)
                    ps2 = psm.tile([D, PAIR], F32, tag="mm", name="ps2")
                    for t in range(2):
                        nc.tensor.matmul(ps2[:, t * TILE:(t + 1) * TILE],
                                         w2, h1[:, t * TILE:(t + 1) * TILE],
                                         start=True, stop=True)
                    nc.scalar.activation(h2[:, hsl], ps2,
                                         mybir.ActivationFunctionType.Lrelu,
                                         bias=b2, scale=1.0, alpha=SLOPE)

                t2 = mid.tile([D, QUAD], BF16, tag="t2", name="t2")
                nc.vector.tensor_mul(t2, h2, ds_bc)
                out_q = io.tile([D, QUAD], BF16, name="out_q")
                nc.vector.tensor_add(out_q, t2, st_q)
                nc.sync.dma_start(outT_d[:, qsl], out_q)

            bc.release()
            mid.release()
            vrows.release()
            io.release()
            res.release()

    nc.compile()
    return nc


def _preprocess(memory, last_update, unique_messages, unique_timestamps,
                static_emb, W1, b1, W2, b2, e_lamb, now_time, unique_sources):
    """Shard + route events + permute; returns (in_maps, perms, NEP)."""
    memory = np.asarray(memory, dtype=np.float32)
    last_update = np.asarray(last_update, dtype=np.float32)
    unique_messages = np.asarray(unique_messages, dtype=np.float32)
    unique_timestamps = np.asarray(unique_timestamps, dtype=np.float32)
    static_emb = np.asarray(static_emb, dtype=np.float32)
    unique_sources = np.asarray(unique_sources)

    owner = unique_sources // S
    order = np.argsort(owner, kind="stable")
    counts = np.bincount(owner, minlength=NCORES)
    starts = np.concatenate([[0], np.cumsum(counts)])
    NEP = int(np.ceil(max(1, counts.max()) / QUAD)) * 2  # even # of pairs
    E_PAD = NEP * PAIR

    w1 = np.asarray(W1, dtype=np.float32)
    w1a = np.ascontiguousarray(w1[:D, :]).astype(NP_BF16)
    w1b = np.ascontiguousarray(w1[D:, :]).astype(NP_BF16)
    w2 = np.ascontiguousarray(np.asarray(W2, dtype=np.float32)).astype(NP_BF16)
    b1c = np.asarray(b1, dtype=np.float32).reshape(D, 1).copy()
    b2c = np.asarray(b2, dtype=np.float32).reshape(D, 1).copy()
    ones = np.ones((1, D), dtype=NP_BF16)

    in_maps = []
    perms = []
    for c in range(NCORES):
        ev_rows = order[starts[c]:starts[c + 1]]
        src_local = unique_sources[ev_rows] - c * S
        E_c = src_local.shape[0]

        is_ev = np.zeros(S, dtype=bool)
        is_ev[src_local] = True
        non_ev = np.nonzero(~is_ev)[0]
        perm = np.concatenate([src_local, non_ev]).astype(np.int64)
        perms.append(perm)

        mem_pad = np.empty((S_PAD, D + 1), dtype=np.float32)
        mem_pad[:S] = memory[c * S:(c + 1) * S][perm]
        mem_pad[S:, :D] = 0.0
        mem_pad[S:, D] = 1.0
        lu_pad = np.zeros(S_PAD, dtype=np.float32)
        lu_pad[:S] = last_update[c * S:(c + 1) * S][perm]
        st_pad = np.zeros((S_PAD, D), dtype=np.float32)
        st_pad[:S] = static_emb[c * S:(c + 1) * S][perm]
        st_pad *= np.float32(e_lamb)   # fold e_lamb into the static table

        msg_full = np.zeros((E_PAD, D + 1), dtype=np.float32)
        msg_full[:E_c] = unique_messages[ev_rows]
        ts_full = np.empty(E_PAD, dtype=np.float32)
        ts_full[:E_c] = unique_timestamps[ev_rows]
        ts_full[E_c:] = lu_pad[E_c:E_PAD]   # identity events: ts = lu, msg = 0

        in_maps.append({
            "msumT": np.ascontiguousarray(mem_pad[:, :D].T).astype(NP_BF16),
            "staticT": np.ascontiguousarray(st_pad.T).astype(NP_BF16),
            "msgT": np.ascontiguousarray(msg_full[:, :D].T).astype(NP_BF16),
            "lu_t": lu_pad.reshape(NP, PAIR).copy(),
            "ts_t": ts_full.reshape(NEP, PAIR).copy(),
            "cnt_t": mem_pad[:, D].reshape(NP, PAIR).copy(),
            "msgc_t": msg_full[:, D].reshape(NEP, PAIR).copy(),
            "w1a": w1a, "w1b": w1b, "w2": w2,
            "b1": b1c, "b2": b2c, "ones": ones,
        })
    return in_maps, perms, NEP


def _run(inputs, trace=False, trace_cores=None):
    in_maps, perms, NEP = _preprocess(**inputs)
    nc = _build(NEP, inputs["e_lamb"], inputs["now_time"])
    res = run_bass_kernel_spmd(nc, in_maps, core_ids=list(range(NCORES)),
                               trace=trace, trace_cores=trace_cores)
    out = np.empty((N_NODES, D), dtype=np.float32)
    for c in range(NCORES):
        out_perm = res.results[c]["outT"].T[:S].astype(np.float32)
        shard = np.empty((S, D), dtype=np.float32)
        shard[perms[c]] = out_perm
        out[c * S:(c + 1) * S] = shard
    return out, res


def kernel(**inputs) -> np.ndarray:
    out, _ = _run(inputs, trace=False)
    return out
